# revision 15
# baseline (speedup 1.0000x reference)
"""DCNv2 (deformable conv v2) TRN2 Bass kernel — data-parallel over batch on
8 NeuronCores (one batch image per core).

Per core: the image is PE-transposed into a channels-last staging tile, then
written into a packed fp16 DRAM scratch where row p holds the 2x2 pixel patch
[p, p+1, p+64, p+65] (4*256 fp16 = 2 KB) via 16 big strided stores (DRAM
stores are whole-tensor WAW-serialized, so few big DMAs reach the first
gather sooner; 133 edge rows pre-zeroed so alpha=0 corners never read NaN).
One `dma_gather` per (kernel-position, 1024 output sites) pulls whole patches
over 4 SWDGE queues. The 4-corner bilinear combine per 128-site group splits
across three engines: corner pair (00,01) via a custom-DVE dual-MAC
(out = in0*s0 + in1*s1, one 1x pass for two corners), corner 10 via an ACT
scale-copy, corner 11 via a DVE scalar_tensor_tensor MAC; the t+v pair-sum
happens for free in the f32 PSUM accumulator of the transposing
identity-matmuls. fp16 GEMMs accumulate over (kernel-position, channel-half)
into PSUM (bias seeded via a K=1 matmul); ACT evicts per site-quarter, one
f32 DMA out.

Stream mapping per (sq, k): stream position i in [0,1024) holds pixel
p = 32*(i%128) + 8*sq + i//128, so the gathered tile G[a, u, :] holds pixel
p = 32a + 8sq + u — the same (partition = p//32) layout as the alpha tiles.
The SWDGE idx tile holds stream position i at (partition i%16, col i//16),
replicated across the 8 Q7 core groups.
"""
import sys
sys.path.insert(0, '/opt/trn_rl_repo')

import numpy as np
import concourse.bass as bass
import concourse.bacc as bacc
import concourse.mybir as mybir
import concourse.tile as tile
from concourse.bass_types import AP
from concourse.bass_utils import run_bass_kernel_spmd
from concourse import library_config, masks

dt = mybir.dt
Alu = mybir.AluOpType
ActFn = mybir.ActivationFunctionType

B = 8
C, H, W = 256, 64, 64
HW = H * W
O = 256
KK = 9
PADR2 = 66                 # packed-patch scratch top pad rows
S2ROWS = PADR2 + HW + 254  # padded so chunked store views stay in bounds;
                           # rows >= PADR2+HW-1 are never gathered
NSQ = 4
NU = 8
NIDX = 1024
F16 = dt.float16
F32 = dt.float32
I16 = dt.int16
I32 = dt.int32

_NC_CACHE = {}

# --- custom DVE op: fused dual MAC (two bilinear corners per pass) --------
# out = in0*s0 + in1*s1 with per-partition scalars s0/s1. One 1x DVE pass
# applies two corner weights, halving the Vector-engine ops of the bilinear
# combine vs chained scalar_tensor_tensor (which has no 2x mode).
import concourse.dve_ops as _dve_ops
from concourse.dve_spec import Spec as _Spec, Src0 as _Src0, Src1 as _Src1, \
    C0 as _C0, C1 as _C1


def _register_dual_mac():
    name = "DCN_DUAL_MAC"
    if name in _dve_ops._SUB_OPCODE_FOR_NAME:
        return next(op for op in _dve_ops.OPS if op.name == name)
    spec = _Spec(
        body=_Src0 * _C0 + _Src1 * _C1,
        reference=lambda in0, in1, s0, s1, imm2: (
            in0.astype(np.float32) * s0 + in1.astype(np.float32) * s1),
    )
    row = _dve_ops._CUSTOM_DVE_ROW_BASE + len(_dve_ops.OPS)
    op = _dve_ops.DveOp(
        name, spec, subdim=False,
        uops_sha={"v3": "f2ac165a27dbafb3", "v4": "49eb47656a95aba3"})
    _dve_ops.OPS.append(op)
    _dve_ops.CUSTOM_DVE_SPECS[name] = spec
    _dve_ops._SUB_OPCODE_FOR_NAME[name] = row
    return op


DUAL_MAC = _register_dual_mac()

# --- queue-aware DMASW semaphore-lane assignment -------------------------
# Tile rotates Pool-engine DMA completion sems over 8 DMASW lanes in
# scheduled order; a lane gets locked to the SWDGE queue that first uses
# it. With 4 SWDGE queues the rotation must keep lane (mod 4) == queue, so
# partition the lanes: queue q uses lanes {q, q+4}.
import concourse.tile_sem_assignment as _tsa

_orig_assign_tick = _tsa.TileClockTick._assign_tick

NQ = 4


def _assign_tick_qaware(self, inst):
    qn = getattr(inst, "queue_num", None)
    if (isinstance(inst, _tsa.DMAInst)
            and inst.engine == mybir.EngineType.Pool and qn is not None):
        if not hasattr(self, "_q_rot"):
            self._q_rot = {}
        r = self._q_rot.get(qn, 0)
        self._q_rot[qn] = (r + 1) % (self.swdge_sem_count // NQ)
        self.next_sw_dma_idx = (qn + NQ * r) % self.swdge_sem_count
    return _orig_assign_tick(self, inst)


_tsa.TileClockTick._assign_tick = _assign_tick_qaware


def build_nc(num_swdge_queues=NQ):
    nc = bacc.Bacc("TRN2", target_bir_lowering=False, debug=True,
                   num_swdge_queues=num_swdge_queues)
    inp = nc.dram_tensor("input", [C, HW], F32, kind="ExternalInput")
    off = nc.dram_tensor("offset", [2 * KK, HW], F32, kind="ExternalInput")
    msk = nc.dram_tensor("mask", [KK, HW], F32, kind="ExternalInput")
    wgt = nc.dram_tensor("weight", [O, C * KK], F32, kind="ExternalInput")
    bia = nc.dram_tensor("bias", [O], F32, kind="ExternalInput")
    out = nc.dram_tensor("out", [O, HW], F32, kind="ExternalOutput")
    scr2 = nc.dram_tensor("scr2", [S2ROWS, 4 * C], F16)

    with tile.TileContext(nc) as tc:
        with tc.tile_pool(name="const", bufs=1) as pc, \
             tc.tile_pool(name="psum_tp", bufs=2, space="PSUM") as ptp, \
             tc.tile_pool(name="psum_mm", bufs=1, space="PSUM") as pmm:

            nc.gpsimd.load_library(library_config.mlp)

            ident = pc.tile([128, 128], F16)
            masks.make_identity(nc, ident[:])

            wt = pc.tile([128, KK, 2, 2, 128], F16)   # [cc, k, ch, oh, o]
            al = pc.tile([128, 4, KK, 32], F32)       # corners 00,01,10,11
            tall = pc.tile([128, NSQ, KK, 64], I16)
            ones = pc.tile([1, 512], F16)
            nc.gpsimd.memset(ones[:], 1.0)
            biash = pc.tile([1, O], F16)
            zt = pc.tile([128, 1024], F16)
            nc.gpsimd.memset(zt[:], 0.0)
            # zero-fill the scr2 rows not fully covered by the 4 slot
            # stores below (slots whose pixel falls outside [0, HW) stay
            # zero): head [0, PADR2) and tail [PADR2+HW-W-1, PADR2+HW)
            nc.sync.dma_start(scr2[0:PADR2, :], zt[0:PADR2, :])
            tail0 = PADR2 + HW - W - 1
            ntail = PADR2 + HW - tail0
            nc.sync.dma_start(scr2[tail0:PADR2 + HW, :], zt[0:ntail, :])

            with tc.tile_pool(name="prep", bufs=1) as pp, \
                 tc.tile_pool(name="prep2", bufs=3) as pp2:
                # --- image -> packed 2x2 patch scratch -------------------
                # scr2 row r slot q holds pixel (r - PADR2 + dlt[q]); the
                # PE-transposed channels-last tiles are stored directly to
                # all four slots (no intermediate channels-last copy).
                imgf = pp.tile([128, 2, HW], F32)
                nc.sync.dma_start(
                    imgf[:], inp[:].rearrange("(ch p) f -> p ch f", p=128))
                imgh = pp.tile([128, 2, HW], F16)
                nc.scalar.activation(imgh[:, 0], imgf[:, 0], ActFn.Copy)
                nc.scalar.activation(imgh[:, 1], imgf[:, 1], ActFn.Copy)
                # transpose the whole image into one staging tile, then
                # write each scr2 slot with a few big strided DMAs (the
                # DRAM writes are whole-tensor WAW-serialized, so fewer,
                # larger stores reach the first gather much sooner)
                stbig = pp.tile([128, 16, 2, 2, 128], F16)
                for pb2 in range(16):
                    tp2f = ptp.tile([128, 1024], F32, tag="tp")
                    tp2 = tp2f[:].bitcast(F16)
                    for t in range(2):
                        for ch in range(2):
                            nc.tensor.transpose(
                                tp2[:, (2 * t + ch) * 128:(2 * t + ch + 1) * 128],
                                imgh[:, ch, (2 * pb2 + t) * 128:(2 * pb2 + t + 1) * 128],
                                ident[:])
                    nc.scalar.activation(
                        stbig[:, pb2].rearrange("p t ch f -> p (t ch f)"),
                        tp2[:, 0:512], ActFn.Copy)
                    if pb2 % 8 == 7:
                        half = pb2 // 8
                        # pixel (2048*half + 256*blk + 128*t + p) -> scr2
                        # row (pixel + PADR2 - dlt), slot q
                        for q, dlt in enumerate([0, 1, W, W + 1]):
                            for t in range(2):
                                r0 = PADR2 - dlt + 2048 * half + 128 * t
                                dst = scr2[r0:r0 + 2048, :].rearrange(
                                    "(blk t2 p) e -> p blk t2 e", p=128, t2=2)
                                nc.sync.dma_start(
                                    dst[:, :, 0, q * 256:(q + 1) * 256],
                                    stbig[:, 8 * half:8 * half + 8, t])

                # --- indices + alphas ---
                offT = pp.tile([128, 2 * KK, 32], F32)
                nc.sync.dma_start(
                    offT[:], off[:].rearrange("c (p j) -> p c j", j=32))
                mT = pp.tile([128, KK, 32], F32)
                nc.sync.dma_start(
                    mT[:], msk[:].rearrange("c (p j) -> p c j", j=32))

                ia32 = pp.tile([128, 1], I32)
                nc.gpsimd.iota(ia32[:], [[1, 1]], base=0, channel_multiplier=1)
                iaf = pp.tile([128, 1], F32)
                nc.vector.tensor_copy(iaf[:], ia32[:])
                hraw = pp.tile([128, 1], F32)
                nc.vector.tensor_scalar(hraw[:], iaf[:], 0.5, None, Alu.mult)
                hi = pp.tile([128, 1], I32)
                nc.vector.tensor_copy(hi[:], hraw[:])
                hf = pp.tile([128, 1], F32)
                nc.vector.tensor_copy(hf[:], hi[:])
                hgt = pp.tile([128, 1], F32)
                nc.vector.tensor_tensor(hgt[:], hf[:], hraw[:], Alu.is_gt)
                h_ap = pp.tile([128, 1], F32)
                nc.vector.tensor_tensor(h_ap[:], hf[:], hgt[:], Alu.subtract)
                am32 = pp.tile([128, 1], F32)
                nc.vector.scalar_tensor_tensor(am32[:], h_ap[:], -2.0, iaf[:],
                                               Alu.mult, Alu.add)
                nc.vector.tensor_scalar(am32[:], am32[:], 32.0, None, Alu.mult)
                jio = pp.tile([128, 32], I32)
                nc.gpsimd.iota(jio[:], [[1, 32]], base=0, channel_multiplier=0)
                jf = pp.tile([128, 32], F32)
                nc.vector.tensor_copy(jf[:], jio[:])
                w32 = pp.tile([128, 32], F32)
                nc.vector.tensor_scalar(w32[:], jf[:], am32[:], None, Alu.add)

                py = pp.tile([128, KK, 32], F32)
                px = pp.tile([128, KK, 32], F32)
                for k in range(KK):
                    ki, kj = k // 3, k % 3
                    nc.vector.tensor_scalar(py[:, k], offT[:, 2 * k], h_ap[:],
                                            float(ki - 1), Alu.add, Alu.add)
                    nc.vector.tensor_scalar(px[:, k], offT[:, 2 * k + 1],
                                            float(kj - 1), None, Alu.add)
                    nc.vector.tensor_tensor(px[:, k], px[:, k], w32[:], Alu.add)

                def floorf(src, flo, frac, nm):
                    ti = pp.tile([128, KK, 32], I32, tag=f"fl_i_{nm}", name=f"fi_{nm}")
                    nc.vector.tensor_copy(ti[:], src)
                    tf = pp.tile([128, KK, 32], F32, tag=f"fl_f_{nm}", name=f"ff_{nm}")
                    nc.vector.tensor_copy(tf[:], ti[:])
                    gt = pp.tile([128, KK, 32], F32, tag=f"fl_g_{nm}", name=f"fg_{nm}")
                    nc.vector.tensor_tensor(gt[:], tf[:], src, Alu.is_gt)
                    nc.vector.tensor_tensor(flo, tf[:], gt[:], Alu.subtract)
                    nc.vector.tensor_tensor(frac, src, flo, Alu.subtract)

                y0 = pp.tile([128, KK, 32], F32)
                ly = pp.tile([128, KK, 32], F32)
                floorf(py[:], y0[:], ly[:], "y")
                x0 = pp.tile([128, KK, 32], F32)
                lx = pp.tile([128, KK, 32], F32)
                floorf(px[:], x0[:], lx[:], "x")

                def cmp2(src, lo, hi_, nm):
                    t1 = pp.tile([128, KK, 32], F32, tag=f"c1_{nm}", name=f"t1_{nm}")
                    nc.vector.tensor_scalar(t1[:], src, lo, None, Alu.is_ge)
                    t2 = pp.tile([128, KK, 32], F32, tag=f"c2_{nm}", name=f"t2_{nm}")
                    nc.vector.tensor_scalar(t2[:], src, hi_, None, Alu.is_le)
                    nc.vector.tensor_tensor(t1[:], t1[:], t2[:], Alu.mult)
                    return t1

                vy0 = cmp2(y0[:], 0.0, 63.0, "vy0")
                vy1 = cmp2(y0[:], -1.0, 62.0, "vy1")
                vx0 = cmp2(x0[:], 0.0, 63.0, "vx0")
                vx1 = cmp2(x0[:], -1.0, 62.0, "vx1")

                oly = pp.tile([128, KK, 32], F32)
                nc.vector.tensor_scalar(oly[:], ly[:], -1.0, 1.0, Alu.mult, Alu.add)
                olx = pp.tile([128, KK, 32], F32)
                nc.vector.tensor_scalar(olx[:], lx[:], -1.0, 1.0, Alu.mult, Alu.add)

                ry0 = pp.tile([128, KK, 32], F32)
                nc.vector.tensor_tensor(ry0[:], oly[:], mT[:], Alu.mult)
                nc.vector.tensor_tensor(ry0[:], ry0[:], vy0[:], Alu.mult)
                ry1 = pp.tile([128, KK, 32], F32)
                nc.vector.tensor_tensor(ry1[:], ly[:], mT[:], Alu.mult)
                nc.vector.tensor_tensor(ry1[:], ry1[:], vy1[:], Alu.mult)
                cx0 = pp.tile([128, KK, 32], F32)
                nc.vector.tensor_tensor(cx0[:], olx[:], vx0[:], Alu.mult)
                cx1 = pp.tile([128, KK, 32], F32)
                nc.vector.tensor_tensor(cx1[:], lx[:], vx1[:], Alu.mult)

                nc.vector.tensor_tensor(al[:, 0], ry0[:], cx0[:], Alu.mult)
                nc.vector.tensor_tensor(al[:, 1], ry0[:], cx1[:], Alu.mult)
                nc.vector.tensor_tensor(al[:, 2], ry1[:], cx0[:], Alu.mult)
                nc.vector.tensor_tensor(al[:, 3], ry1[:], cx1[:], Alu.mult)

                cy0 = pp.tile([128, KK, 32], F32)
                nc.vector.tensor_scalar(cy0[:], y0[:], -1.0, 63.0, Alu.max, Alu.min)
                cxc = pp.tile([128, KK, 32], F32)
                nc.vector.tensor_scalar(cxc[:], x0[:], -1.0, 63.0, Alu.max, Alu.min)
                cxp0 = pp.tile([128, KK, 32], F32)
                nc.vector.tensor_scalar(cxp0[:], cxc[:], float(PADR2), None, Alu.add)
                i0t = pp.tile([128, KK, 32], F32)
                nc.vector.scalar_tensor_tensor(i0t[:], cy0[:], float(W),
                                               cxp0[:], Alu.mult, Alu.add)

                st16 = pp.tile([16, 8, KK, 32], F32)
                for v in range(8):
                    nc.sync.dma_start(st16[:, v], i0t[16 * v:16 * v + 16])
                for sq in range(NSQ):
                    dst = tall[0:16, sq, :, :].rearrange(
                        "p k (u v) -> p v k u", v=8)
                    src = st16[:, :, :, 8 * sq:8 * sq + 8]
                    nc.vector.tensor_copy(dst, src)
                for g in range(1, 8):
                    nc.sync.dma_start(tall[16 * g:16 * g + 16], tall[0:16])

                # --- weights -> lhsT tiles ---
                wldf = pp.tile([128, 2, C * KK], F32)
                nc.sync.dma_start(
                    wldf[:], wgt[:].rearrange("(oh p) ck -> p oh ck", p=128))
                wldh = pp.tile([128, 2, C * KK], F16)
                nc.scalar.activation(wldh[:, 0], wldf[:, 0], ActFn.Copy)
                nc.scalar.activation(wldh[:, 1], wldf[:, 1], ActFn.Copy)
                for k in range(KK):
                    tpwf = ptp.tile([128, 1024], F32, tag="tp")
                    tpw = tpwf[:].bitcast(F16)
                    wview = wldh[:].rearrange("p oh (c k) -> p oh c k", k=KK)
                    for ch in range(2):
                        for oh in range(2):
                            nc.tensor.transpose(
                                tpw[:, (ch * 2 + oh) * 128:(ch * 2 + oh + 1) * 128],
                                wview[:, oh, ch * 128:(ch + 1) * 128, k], ident[:])
                    nc.scalar.activation(
                        wt[:, k].rearrange("p ch oh f -> p (ch oh f)"),
                        tpw[:, 0:512], ActFn.Copy)

                biasf = pp.tile([1, O], F32)
                nc.sync.dma_start(biasf[:], bia[:].rearrange("(a f) -> a f", a=1))
                nc.vector.tensor_copy(biash[:], biasf[:])

            # ---------------- main loop ----------------
            with tc.tile_pool(name="mainp", bufs=1) as pm, \
                 tc.tile_pool(name="gpool", bufs=6) as pg, \
                 tc.tile_pool(name="tvp", bufs=3) as ptv, \
                 tc.tile_pool(name="colsp", bufs=3) as pcl:
                outS = pm.tile([128, 2, HW], F32)
                for sq in range(NSQ):
                    po = [pmm.tile([128, NIDX], F32, tag=f"mo{oh}",
                                   name=f"po{oh}_{sq}")
                          for oh in range(2)]
                    for oh in range(2):
                        for n2 in range(2):
                            nc.tensor.matmul(
                                po[oh][:, n2 * 512:(n2 + 1) * 512],
                                biash[0:1, oh * 128:(oh + 1) * 128],
                                ones[0:1, :], start=True, stop=False)
                    for k in range(KK):
                        gp = pg.tile([128, NU, 4 * C], F16, tag="g",
                                     name=f"g_{sq}_{k}")
                        nc.gpsimd.dma_gather(gp[:], scr2[:], tall[:, sq, k, :],
                                             NIDX, NIDX, 4 * C,
                                             single_packet=False,
                                             queue_num=k % num_swdge_queues)
                        tv = ptv.tile([128, 2, NU, C], F16, tag="tv",
                                      name=f"tv_{sq}_{k}")
                        for u in range(NU):
                            j = 8 * sq + u
                            nc.scalar.activation(
                                tv[:, 1, u], gp[:, u, 2 * C:3 * C], ActFn.Copy,
                                scale=al[:, 2, k, j:j + 1])
                        for u in range(NU):
                            j = 8 * sq + u
                            nc.vector._custom_dve(
                                DUAL_MAC, out=tv[:, 0, u],
                                in0=gp[:, u, 0:C], in1=gp[:, u, C:2 * C],
                                s0=al[:, 0, k, j:j + 1], s1=al[:, 1, k, j:j + 1])
                        for u in range(NU):
                            j = 8 * sq + u
                            nc.vector.scalar_tensor_tensor(
                                tv[:, 1, u], gp[:, u, 3 * C:4 * C],
                                al[:, 3, k, j:j + 1], tv[:, 1, u],
                                Alu.mult, Alu.add)
                        colsq = pcl.tile([128, 2, NIDX], F16, tag="cols",
                                         name=f"cols_{sq}_{k}")
                        for ch in range(2):
                            # pair-sum t+v happens in the f32 PSUM
                            # accumulator of the transposing matmuls
                            tpp = ptp.tile([128, 1024], F32, tag="tp",
                                           name=f"tp_{sq}_{k}_{ch}")
                            for u in range(NU):
                                for half in range(2):
                                    nc.tensor.matmul(
                                        tpp[:, u * 128:(u + 1) * 128],
                                        tv[:, half, u, ch * 128:(ch + 1) * 128],
                                        ident[:],
                                        start=(half == 0), stop=(half == 1))
                            nc.scalar.activation(colsq[:, ch, :],
                                                 tpp[:, 0:1024], ActFn.Copy)
                        for oh in range(2):
                            for ch in range(2):
                                for n2 in range(2):
                                    nc.tensor.matmul(
                                        po[oh][:, n2 * 512:(n2 + 1) * 512],
                                        wt[:, k, ch, oh, :],
                                        colsq[:, ch, n2 * 512:(n2 + 1) * 512],
                                        start=False,
                                        stop=(k == KK - 1 and ch == 1))
                    for oh in range(2):
                        dst = outS[:, oh, :].rearrange(
                            "p (a j) -> p j a", j=32)[:, 8 * sq:8 * sq + 8, :]
                        src = po[oh][:].rearrange("p (u a) -> p u a", u=8)
                        nc.scalar.activation(dst, src, ActFn.Copy)
                nc.sync.dma_start(
                    out[:].rearrange("(oh p) f -> p oh f", p=128), outS[:])
    nc.compile()
    return nc


def _get_nc():
    if "nc" not in _NC_CACHE:
        _NC_CACHE["nc"] = build_nc()
    return _NC_CACHE["nc"]


def kernel(**inputs):
    inp = np.ascontiguousarray(np.asarray(inputs["input"], dtype=np.float32))
    off = np.ascontiguousarray(np.asarray(inputs["offset"], dtype=np.float32))
    msk = np.ascontiguousarray(np.asarray(inputs["mask"], dtype=np.float32))
    wgt = np.ascontiguousarray(np.asarray(inputs["weight"], dtype=np.float32))
    bia = np.ascontiguousarray(np.asarray(inputs["bias"], dtype=np.float32))
    assert inp.shape == (B, C, H, W)

    wflat = wgt.reshape(O, C * KK)
    in_maps = []
    for b in range(B):
        in_maps.append({
            "input": inp[b].reshape(C, HW),
            "offset": off[b].reshape(2 * KK, HW),
            "mask": msk[b].reshape(KK, HW),
            "weight": wflat,
            "bias": bia,
        })
    nc = _get_nc()
    res = run_bass_kernel_spmd(nc, in_maps, list(range(B)))
    out = np.stack([res.results[b]["out"].reshape(O, H, W) for b in range(B)])
    return out.astype(np.float32)


if __name__ == "__main__":
    rng = np.random.default_rng(0)
    ins = {
        "input": rng.standard_normal((B, C, H, W)).astype(np.float32),
        "offset": rng.standard_normal((B, 2 * KK, H, W)).astype(np.float32),
        "mask": rng.random((B, KK, H, W)).astype(np.float32),
        "weight": rng.uniform(-1 / 48, 1 / 48, (O, C, 3, 3)).astype(np.float32),
        "bias": np.zeros((O,), np.float32),
    }
    o = kernel(**ins)
    print("kernel ran, out shape", o.shape, "finite:", np.isfinite(o).all())


# revision 16
# speedup vs baseline: 1.0162x; 1.0162x over previous
"""DCNv2 (deformable conv v2) TRN2 Bass kernel — data-parallel over batch on
8 NeuronCores (one batch image per core).

Per core: the image is PE-transposed into a channels-last staging tile, then
written into a packed fp16 DRAM scratch where row p holds the 2x2 pixel patch
[p, p+1, p+64, p+65] (4*256 fp16 = 2 KB) via 16 big strided stores (DRAM
stores are whole-tensor WAW-serialized, so few big DMAs reach the first
gather sooner; 133 edge rows pre-zeroed so alpha=0 corners never read NaN).
One `dma_gather` per (kernel-position, 1024 output sites) pulls whole patches
over 4 SWDGE queues. The 4-corner bilinear combine per 128-site group splits
across three engines: corner pair (00,01) via a custom-DVE dual-MAC
(out = in0*s0 + in1*s1, one 1x pass for two corners), corner 10 via an ACT
scale-copy, corner 11 via a DVE scalar_tensor_tensor MAC; the t+v pair-sum
happens for free in the f32 PSUM accumulator of the transposing
identity-matmuls. fp16 GEMMs accumulate over (kernel-position, channel-half)
into PSUM (bias seeded via a K=1 matmul); ACT evicts per site-quarter, one
f32 DMA out.

Stream mapping per (sq, k): stream position i in [0,1024) holds pixel
p = 32*(i%128) + 8*sq + i//128, so the gathered tile G[a, u, :] holds pixel
p = 32a + 8sq + u — the same (partition = p//32) layout as the alpha tiles.
The SWDGE idx tile holds stream position i at (partition i%16, col i//16),
replicated across the 8 Q7 core groups.
"""
import sys
sys.path.insert(0, '/opt/trn_rl_repo')

import numpy as np
import concourse.bass as bass
import concourse.bacc as bacc
import concourse.mybir as mybir
import concourse.tile as tile
from concourse.bass_types import AP
from concourse.bass_utils import run_bass_kernel_spmd
from concourse import library_config, masks

dt = mybir.dt
Alu = mybir.AluOpType
ActFn = mybir.ActivationFunctionType

B = 8
C, H, W = 256, 64, 64
HW = H * W
O = 256
KK = 9
PADR2 = 66                 # packed-patch scratch top pad rows
S2ROWS = PADR2 + HW + 254  # padded so chunked store views stay in bounds;
                           # rows >= PADR2+HW-1 are never gathered
NSQ = 4
NU = 8
NIDX = 1024
F16 = dt.float16
F32 = dt.float32
I16 = dt.int16
I32 = dt.int32

_NC_CACHE = {}

# --- custom DVE op: fused dual MAC (two bilinear corners per pass) --------
# out = in0*s0 + in1*s1 with per-partition scalars s0/s1. One 1x DVE pass
# applies two corner weights, halving the Vector-engine ops of the bilinear
# combine vs chained scalar_tensor_tensor (which has no 2x mode).
import concourse.dve_ops as _dve_ops
from concourse.dve_spec import Spec as _Spec, Src0 as _Src0, Src1 as _Src1, \
    C0 as _C0, C1 as _C1


def _register_dual_mac():
    name = "DCN_DUAL_MAC"
    if name in _dve_ops._SUB_OPCODE_FOR_NAME:
        return next(op for op in _dve_ops.OPS if op.name == name)
    spec = _Spec(
        body=_Src0 * _C0 + _Src1 * _C1,
        reference=lambda in0, in1, s0, s1, imm2: (
            in0.astype(np.float32) * s0 + in1.astype(np.float32) * s1),
    )
    row = _dve_ops._CUSTOM_DVE_ROW_BASE + len(_dve_ops.OPS)
    op = _dve_ops.DveOp(
        name, spec, subdim=False,
        uops_sha={"v3": "f2ac165a27dbafb3", "v4": "49eb47656a95aba3"})
    _dve_ops.OPS.append(op)
    _dve_ops.CUSTOM_DVE_SPECS[name] = spec
    _dve_ops._SUB_OPCODE_FOR_NAME[name] = row
    return op


DUAL_MAC = _register_dual_mac()

# --- queue-aware DMASW semaphore-lane assignment -------------------------
# Tile rotates Pool-engine DMA completion sems over 8 DMASW lanes in
# scheduled order; a lane gets locked to the SWDGE queue that first uses
# it. With 4 SWDGE queues the rotation must keep lane (mod 4) == queue, so
# partition the lanes: queue q uses lanes {q, q+4}.
import concourse.tile_sem_assignment as _tsa

_orig_assign_tick = _tsa.TileClockTick._assign_tick

NQ = 4


def _assign_tick_qaware(self, inst):
    qn = getattr(inst, "queue_num", None)
    if (isinstance(inst, _tsa.DMAInst)
            and inst.engine == mybir.EngineType.Pool and qn is not None):
        if not hasattr(self, "_q_rot"):
            self._q_rot = {}
        r = self._q_rot.get(qn, 0)
        self._q_rot[qn] = (r + 1) % (self.swdge_sem_count // NQ)
        self.next_sw_dma_idx = (qn + NQ * r) % self.swdge_sem_count
    return _orig_assign_tick(self, inst)


_tsa.TileClockTick._assign_tick = _assign_tick_qaware


def build_nc(num_swdge_queues=NQ):
    nc = bacc.Bacc("TRN2", target_bir_lowering=False, debug=True,
                   num_swdge_queues=num_swdge_queues)
    inp = nc.dram_tensor("input", [C, HW], F32, kind="ExternalInput")
    off = nc.dram_tensor("offset", [2 * KK, HW], F32, kind="ExternalInput")
    msk = nc.dram_tensor("mask", [KK, HW], F32, kind="ExternalInput")
    wgt = nc.dram_tensor("weight", [O, C * KK], F32, kind="ExternalInput")
    bia = nc.dram_tensor("bias", [O], F32, kind="ExternalInput")
    out = nc.dram_tensor("out", [O, HW], F32, kind="ExternalOutput")
    scr2 = nc.dram_tensor("scr2", [S2ROWS, 4 * C], F16)

    with tile.TileContext(nc) as tc:
        with tc.tile_pool(name="const", bufs=1) as pc, \
             tc.tile_pool(name="psum_tp", bufs=2, space="PSUM") as ptp, \
             tc.tile_pool(name="psum_mm", bufs=1, space="PSUM") as pmm:

            nc.gpsimd.load_library(library_config.mlp)

            ident = pc.tile([128, 128], F16)
            masks.make_identity(nc, ident[:])

            wt = pc.tile([128, KK, 2, 2, 128], F16)   # [cc, k, ch, oh, o]
            al = pc.tile([128, 4, KK, 32], F32)       # corners 00,01,10,11
            tall = pc.tile([128, NSQ, KK, 64], I16)
            ones = pc.tile([1, 512], F16)
            nc.gpsimd.memset(ones[:], 1.0)
            biash = pc.tile([1, O], F16)
            zt = pc.tile([128, 1024], F16)
            nc.gpsimd.memset(zt[:], 0.0)
            # zero-fill the scr2 rows not fully covered by the 4 slot
            # stores below (slots whose pixel falls outside [0, HW) stay
            # zero): head [0, PADR2) and tail [PADR2+HW-W-1, PADR2+HW)
            nc.sync.dma_start(scr2[0:PADR2, :], zt[0:PADR2, :])
            tail0 = PADR2 + HW - W - 1
            ntail = PADR2 + HW - tail0
            nc.sync.dma_start(scr2[tail0:PADR2 + HW, :], zt[0:ntail, :])

            with tc.tile_pool(name="prep", bufs=1) as pp, \
                 tc.tile_pool(name="prep2", bufs=3) as pp2:
                # --- image -> packed 2x2 patch scratch -------------------
                # scr2 row r slot q holds pixel (r - PADR2 + dlt[q]); the
                # PE-transposed channels-last tiles are stored directly to
                # all four slots (no intermediate channels-last copy).
                imgf = pp.tile([128, 2, HW], F32)
                nc.sync.dma_start(
                    imgf[:], inp[:].rearrange("(ch p) f -> p ch f", p=128))
                imgh = pp.tile([128, 2, HW], F16)
                nc.scalar.activation(imgh[:, 0], imgf[:, 0], ActFn.Copy)
                nc.scalar.activation(imgh[:, 1], imgf[:, 1], ActFn.Copy)
                # transpose the whole image into one staging tile, then
                # write each scr2 slot with a few big strided DMAs (the
                # DRAM writes are whole-tensor WAW-serialized, so fewer,
                # larger stores reach the first gather much sooner)
                stbig = pp.tile([128, 16, 2, 2, 128], F16)
                for pb2 in range(16):
                    tp2f = ptp.tile([128, 1024], F32, tag="tp")
                    tp2 = tp2f[:].bitcast(F16)
                    for t in range(2):
                        for ch in range(2):
                            nc.tensor.transpose(
                                tp2[:, (2 * t + ch) * 128:(2 * t + ch + 1) * 128],
                                imgh[:, ch, (2 * pb2 + t) * 128:(2 * pb2 + t + 1) * 128],
                                ident[:])
                    nc.scalar.activation(
                        stbig[:, pb2].rearrange("p t ch f -> p (t ch f)"),
                        tp2[:, 0:512], ActFn.Copy)
                    if pb2 % 8 == 7:
                        half = pb2 // 8
                        # pixel (2048*half + 256*blk + 128*t + p) -> scr2
                        # row (pixel + PADR2 - dlt), slot q
                        for q, dlt in enumerate([0, 1, W, W + 1]):
                            for t in range(2):
                                r0 = PADR2 - dlt + 2048 * half + 128 * t
                                dst = scr2[r0:r0 + 2048, :].rearrange(
                                    "(blk t2 p) e -> p blk t2 e", p=128, t2=2)
                                nc.sync.dma_start(
                                    dst[:, :, 0, q * 256:(q + 1) * 256],
                                    stbig[:, 8 * half:8 * half + 8, t])

                # --- indices + alphas ---
                offT = pp.tile([128, 2 * KK, 32], F32)
                nc.sync.dma_start(
                    offT[:], off[:].rearrange("c (p j) -> p c j", j=32))
                mT = pp.tile([128, KK, 32], F32)
                nc.sync.dma_start(
                    mT[:], msk[:].rearrange("c (p j) -> p c j", j=32))

                ia32 = pp.tile([128, 1], I32)
                nc.gpsimd.iota(ia32[:], [[1, 1]], base=0, channel_multiplier=1)
                iaf = pp.tile([128, 1], F32)
                nc.vector.tensor_copy(iaf[:], ia32[:])
                hraw = pp.tile([128, 1], F32)
                nc.vector.tensor_scalar(hraw[:], iaf[:], 0.5, None, Alu.mult)
                hi = pp.tile([128, 1], I32)
                nc.vector.tensor_copy(hi[:], hraw[:])
                hf = pp.tile([128, 1], F32)
                nc.vector.tensor_copy(hf[:], hi[:])
                hgt = pp.tile([128, 1], F32)
                nc.vector.tensor_tensor(hgt[:], hf[:], hraw[:], Alu.is_gt)
                h_ap = pp.tile([128, 1], F32)
                nc.vector.tensor_tensor(h_ap[:], hf[:], hgt[:], Alu.subtract)
                am32 = pp.tile([128, 1], F32)
                nc.vector.scalar_tensor_tensor(am32[:], h_ap[:], -2.0, iaf[:],
                                               Alu.mult, Alu.add)
                nc.vector.tensor_scalar(am32[:], am32[:], 32.0, None, Alu.mult)
                jio = pp.tile([128, 32], I32)
                nc.gpsimd.iota(jio[:], [[1, 32]], base=0, channel_multiplier=0)
                jf = pp.tile([128, 32], F32)
                nc.vector.tensor_copy(jf[:], jio[:])
                w32 = pp.tile([128, 32], F32)
                nc.vector.tensor_scalar(w32[:], jf[:], am32[:], None, Alu.add)

                py = pp.tile([128, KK, 32], F32)
                px = pp.tile([128, KK, 32], F32)
                for k in range(KK):
                    ki, kj = k // 3, k % 3
                    nc.vector.tensor_scalar(py[:, k], offT[:, 2 * k], h_ap[:],
                                            float(ki - 1), Alu.add, Alu.add)
                    nc.vector.tensor_scalar(px[:, k], offT[:, 2 * k + 1],
                                            float(kj - 1), None, Alu.add)
                    nc.vector.tensor_tensor(px[:, k], px[:, k], w32[:], Alu.add)

                def floorf(src, flo, frac, nm):
                    ti = pp.tile([128, KK, 32], I32, tag=f"fl_i_{nm}", name=f"fi_{nm}")
                    nc.vector.tensor_copy(ti[:], src)
                    tf = pp.tile([128, KK, 32], F32, tag=f"fl_f_{nm}", name=f"ff_{nm}")
                    nc.vector.tensor_copy(tf[:], ti[:])
                    gt = pp.tile([128, KK, 32], F32, tag=f"fl_g_{nm}", name=f"fg_{nm}")
                    nc.vector.tensor_tensor(gt[:], tf[:], src, Alu.is_gt)
                    nc.vector.tensor_tensor(flo, tf[:], gt[:], Alu.subtract)
                    nc.vector.tensor_tensor(frac, src, flo, Alu.subtract)

                y0 = pp.tile([128, KK, 32], F32)
                ly = pp.tile([128, KK, 32], F32)
                floorf(py[:], y0[:], ly[:], "y")
                x0 = pp.tile([128, KK, 32], F32)
                lx = pp.tile([128, KK, 32], F32)
                floorf(px[:], x0[:], lx[:], "x")

                def cmp2(src, lo, hi_, nm):
                    t1 = pp.tile([128, KK, 32], F32, tag=f"c1_{nm}", name=f"t1_{nm}")
                    nc.vector.tensor_scalar(t1[:], src, lo, None, Alu.is_ge)
                    t2 = pp.tile([128, KK, 32], F32, tag=f"c2_{nm}", name=f"t2_{nm}")
                    nc.vector.tensor_scalar(t2[:], src, hi_, None, Alu.is_le)
                    nc.vector.tensor_tensor(t1[:], t1[:], t2[:], Alu.mult)
                    return t1

                vy0 = cmp2(y0[:], 0.0, 63.0, "vy0")
                vy1 = cmp2(y0[:], -1.0, 62.0, "vy1")
                vx0 = cmp2(x0[:], 0.0, 63.0, "vx0")
                vx1 = cmp2(x0[:], -1.0, 62.0, "vx1")

                oly = pp.tile([128, KK, 32], F32)
                nc.vector.tensor_scalar(oly[:], ly[:], -1.0, 1.0, Alu.mult, Alu.add)
                olx = pp.tile([128, KK, 32], F32)
                nc.vector.tensor_scalar(olx[:], lx[:], -1.0, 1.0, Alu.mult, Alu.add)

                ry0 = pp.tile([128, KK, 32], F32)
                nc.vector.tensor_tensor(ry0[:], oly[:], mT[:], Alu.mult)
                nc.vector.tensor_tensor(ry0[:], ry0[:], vy0[:], Alu.mult)
                ry1 = pp.tile([128, KK, 32], F32)
                nc.vector.tensor_tensor(ry1[:], ly[:], mT[:], Alu.mult)
                nc.vector.tensor_tensor(ry1[:], ry1[:], vy1[:], Alu.mult)
                cx0 = pp.tile([128, KK, 32], F32)
                nc.vector.tensor_tensor(cx0[:], olx[:], vx0[:], Alu.mult)
                cx1 = pp.tile([128, KK, 32], F32)
                nc.vector.tensor_tensor(cx1[:], lx[:], vx1[:], Alu.mult)

                nc.vector.tensor_tensor(al[:, 0], ry0[:], cx0[:], Alu.mult)
                nc.vector.tensor_tensor(al[:, 1], ry0[:], cx1[:], Alu.mult)
                nc.vector.tensor_tensor(al[:, 2], ry1[:], cx0[:], Alu.mult)
                nc.vector.tensor_tensor(al[:, 3], ry1[:], cx1[:], Alu.mult)

                cy0 = pp.tile([128, KK, 32], F32)
                nc.vector.tensor_scalar(cy0[:], y0[:], -1.0, 63.0, Alu.max, Alu.min)
                cxc = pp.tile([128, KK, 32], F32)
                nc.vector.tensor_scalar(cxc[:], x0[:], -1.0, 63.0, Alu.max, Alu.min)
                cxp0 = pp.tile([128, KK, 32], F32)
                nc.vector.tensor_scalar(cxp0[:], cxc[:], float(PADR2), None, Alu.add)
                i0t = pp.tile([128, KK, 32], F32)
                nc.vector.scalar_tensor_tensor(i0t[:], cy0[:], float(W),
                                               cxp0[:], Alu.mult, Alu.add)

                st16 = pp.tile([16, 8, KK, 32], F32)
                for v in range(8):
                    nc.sync.dma_start(st16[:, v], i0t[16 * v:16 * v + 16])
                for sq in range(NSQ):
                    dst = tall[0:16, sq, :, :].rearrange(
                        "p k (u v) -> p v k u", v=8)
                    src = st16[:, :, :, 8 * sq:8 * sq + 8]
                    nc.vector.tensor_copy(dst, src)
                for g in range(1, 8):
                    nc.sync.dma_start(tall[16 * g:16 * g + 16], tall[0:16])

                # --- weights -> lhsT tiles ---
                wldf = pp.tile([128, 2, C * KK], F32)
                nc.sync.dma_start(
                    wldf[:], wgt[:].rearrange("(oh p) ck -> p oh ck", p=128))
                wldh = pp.tile([128, 2, C * KK], F16)
                nc.scalar.activation(wldh[:, 0], wldf[:, 0], ActFn.Copy)
                nc.scalar.activation(wldh[:, 1], wldf[:, 1], ActFn.Copy)
                for k in range(KK):
                    tpwf = ptp.tile([128, 1024], F32, tag="tp")
                    tpw = tpwf[:].bitcast(F16)
                    wview = wldh[:].rearrange("p oh (c k) -> p oh c k", k=KK)
                    for ch in range(2):
                        for oh in range(2):
                            nc.tensor.transpose(
                                tpw[:, (ch * 2 + oh) * 128:(ch * 2 + oh + 1) * 128],
                                wview[:, oh, ch * 128:(ch + 1) * 128, k], ident[:])
                    nc.scalar.activation(
                        wt[:, k].rearrange("p ch oh f -> p (ch oh f)"),
                        tpw[:, 0:512], ActFn.Copy)

                biasf = pp.tile([1, O], F32)
                nc.sync.dma_start(biasf[:], bia[:].rearrange("(a f) -> a f", a=1))
                nc.vector.tensor_copy(biash[:], biasf[:])

            # ---------------- main loop ----------------
            with tc.tile_pool(name="mainp", bufs=1) as pm, \
                 tc.tile_pool(name="gpool", bufs=5) as pg, \
                 tc.tile_pool(name="tvp", bufs=3) as ptv, \
                 tc.tile_pool(name="colsp", bufs=3) as pcl:
                outS = pm.tile([128, 2, HW], F32)
                for sq in range(NSQ):
                    po = [pmm.tile([128, NIDX], F32, tag=f"mo{oh}",
                                   name=f"po{oh}_{sq}")
                          for oh in range(2)]
                    for oh in range(2):
                        for n2 in range(2):
                            nc.tensor.matmul(
                                po[oh][:, n2 * 512:(n2 + 1) * 512],
                                biash[0:1, oh * 128:(oh + 1) * 128],
                                ones[0:1, :], start=True, stop=False)
                    for k in range(KK):
                        gp = pg.tile([128, NU, 4 * C], F16, tag="g",
                                     name=f"g_{sq}_{k}")
                        nc.gpsimd.dma_gather(gp[:], scr2[:], tall[:, sq, k, :],
                                             NIDX, NIDX, 4 * C,
                                             single_packet=False,
                                             queue_num=k % num_swdge_queues)
                        tv = ptv.tile([128, 2, NU, C], F16, tag="tv",
                                      name=f"tv_{sq}_{k}")
                        for u in range(NU):
                            j = 8 * sq + u
                            nc.scalar.activation(
                                tv[:, 1, u], gp[:, u, 2 * C:3 * C], ActFn.Copy,
                                scale=al[:, 2, k, j:j + 1])
                        for u in range(NU):
                            j = 8 * sq + u
                            nc.vector._custom_dve(
                                DUAL_MAC, out=tv[:, 0, u],
                                in0=gp[:, u, 0:C], in1=gp[:, u, C:2 * C],
                                s0=al[:, 0, k, j:j + 1], s1=al[:, 1, k, j:j + 1])
                        for u in range(NU):
                            j = 8 * sq + u
                            nc.vector.scalar_tensor_tensor(
                                tv[:, 1, u], gp[:, u, 3 * C:4 * C],
                                al[:, 3, k, j:j + 1], tv[:, 1, u],
                                Alu.mult, Alu.add)
                        colsq = pcl.tile([128, 2, NIDX], F16, tag="cols",
                                         name=f"cols_{sq}_{k}")
                        for ch in range(2):
                            # pair-sum t+v happens in the f32 PSUM
                            # accumulator of the transposing matmuls
                            tpp = ptp.tile([128, 1024], F32, tag="tp",
                                           name=f"tp_{sq}_{k}_{ch}")
                            for u in range(NU):
                                for half in range(2):
                                    nc.tensor.matmul(
                                        tpp[:, u * 128:(u + 1) * 128],
                                        tv[:, half, u, ch * 128:(ch + 1) * 128],
                                        ident[:],
                                        start=(half == 0), stop=(half == 1))
                            nc.scalar.activation(colsq[:, ch, :],
                                                 tpp[:, 0:1024], ActFn.Copy)
                        for oh in range(2):
                            for ch in range(2):
                                for n2 in range(2):
                                    nc.tensor.matmul(
                                        po[oh][:, n2 * 512:(n2 + 1) * 512],
                                        wt[:, k, ch, oh, :],
                                        colsq[:, ch, n2 * 512:(n2 + 1) * 512],
                                        start=False,
                                        stop=(k == KK - 1 and ch == 1))
                    for oh in range(2):
                        dst = outS[:, oh, :].rearrange(
                            "p (a j) -> p j a", j=32)[:, 8 * sq:8 * sq + 8, :]
                        src = po[oh][:].rearrange("p (u a) -> p u a", u=8)
                        nc.scalar.activation(dst, src, ActFn.Copy)
                nc.sync.dma_start(
                    out[:].rearrange("(oh p) f -> p oh f", p=128), outS[:])
    nc.compile()
    return nc


def _get_nc():
    if "nc" not in _NC_CACHE:
        _NC_CACHE["nc"] = build_nc()
    return _NC_CACHE["nc"]


def kernel(**inputs):
    inp = np.ascontiguousarray(np.asarray(inputs["input"], dtype=np.float32))
    off = np.ascontiguousarray(np.asarray(inputs["offset"], dtype=np.float32))
    msk = np.ascontiguousarray(np.asarray(inputs["mask"], dtype=np.float32))
    wgt = np.ascontiguousarray(np.asarray(inputs["weight"], dtype=np.float32))
    bia = np.ascontiguousarray(np.asarray(inputs["bias"], dtype=np.float32))
    assert inp.shape == (B, C, H, W)

    wflat = wgt.reshape(O, C * KK)
    in_maps = []
    for b in range(B):
        in_maps.append({
            "input": inp[b].reshape(C, HW),
            "offset": off[b].reshape(2 * KK, HW),
            "mask": msk[b].reshape(KK, HW),
            "weight": wflat,
            "bias": bia,
        })
    nc = _get_nc()
    res = run_bass_kernel_spmd(nc, in_maps, list(range(B)))
    out = np.stack([res.results[b]["out"].reshape(O, H, W) for b in range(B)])
    return out.astype(np.float32)


if __name__ == "__main__":
    rng = np.random.default_rng(0)
    ins = {
        "input": rng.standard_normal((B, C, H, W)).astype(np.float32),
        "offset": rng.standard_normal((B, 2 * KK, H, W)).astype(np.float32),
        "mask": rng.random((B, KK, H, W)).astype(np.float32),
        "weight": rng.uniform(-1 / 48, 1 / 48, (O, C, 3, 3)).astype(np.float32),
        "bias": np.zeros((O,), np.float32),
    }
    o = kernel(**ins)
    print("kernel ran, out shape", o.shape, "finite:", np.isfinite(o).all())
